# revision 55
# baseline (speedup 1.0000x reference)
"""Distributed Trainium2 kernel for nn_ADLoss_38354057953796 (v5).

Strategy: shard P and C along the FRAME axis (F=4096 -> 512 per core).
Each core sees the full batch for its frame slice, so per-class segment
sums are local PE matmuls; only tiny AllReduces cross cores.

v5 (from v4 trace analysis, 267us):
  * The CC stack has a fixed startup barrier (~45-58us starting at
    ~21us) that gates the first collective, and ARs serialize at
    ~8us each on one CC stream.  Chunked AR1 therefore only added
    serial CC time; v5 uses ONE warmup (absorbs the first-AR penalty
    during the load) + ONE 16KB AR1 of all per-(b,h) sq norms.
  * P stays RAW in SBUF (bf16); invn is folded into the phase-B lhsT
    (ohn = onehot*invn) and into phase D fused ops / combine weights.
  * Phase B runs h-major and the RAW C_upd gather table is written in
    TWO head-halves, so phase D-A (heads 0-3) starts while phase B is
    still finishing heads 4-7.
  * Phase D head slots split by measured cost: STT on DVE (~0.95us),
    raw mul on gpsimd (~1.36us) or DVE (~0.49us) + accum on ACT
    (~0.99us).  gpsimd scalar_tensor_tensor is broken on this stack
    (device hang) - only plain tensor ops there.
  * gram: 4 packed jobs (128 partitions, bottom block = emas rolled by
    4 heads), DVE muls + DVE job-reduces, issued between the A and B
    phase-D loops.
  * Collectives: warmup, AR1 (sq), AR2a (csq 1KB), AR3 (ips2+gram in
    one 8.7KB bin).
"""

import sys
import numpy as np

for _p in ("/opt/trn_rl_repo",):
    if _p not in sys.path:
        sys.path.insert(0, _p)

B, H, F, CLS = 1024, 8, 4096, 64
M = 8            # cores
FL = F // M      # local frame slice = 512
NT = 8           # batch tiles
PT = 128         # partitions per tile
ETA = 0.1
DELTA_BETWEEN = 1.0

TRACE = False
LAST_EXEC_NS = None
LAST_RESULTS = None

# packed gram jobs: (d_top, d_bottom=d+4) pairs; bottom uses emas rolled
# by 4 heads
_GJOBS = [(0, 8, 0), (1, 7, 8), (2, 6, 15), (3, 5, 21)]  # (d, n, col_base)
_GCOLS = 26

# phase-D head slots: heads 0-3 gather from table A, 4-7 from table B
D_STT_DVE = (0, 1, 2)        # fused (pb*invn)*g + accum on DVE
D_MUL_GPS = (3, 4, 5)        # raw mul on gpsimd, accum on ACT
D_MUL_DVE = (6, 7)           # raw mul on DVE, accum on ACT
_HALVES = ((0, 1, 2, 3), (4, 5, 6, 7))


class _StageCut(Exception):
    pass


def _build(labels, delta_within, stage=99):
    import concourse.bass as bass
    import concourse.tile as tile
    from concourse import mybir
    import ml_dtypes

    f32 = mybir.dt.float32
    bf16 = mybir.dt.bfloat16
    i32 = mybir.dt.int32
    AF = mybir.ActivationFunctionType
    OP = mybir.AluOpType
    AX = mybir.AxisListType
    RG = [list(range(M))]

    labels = np.asarray(labels).astype(np.int64).reshape(B)
    dw = np.asarray(delta_within).astype(np.float32).reshape(CLS)

    counts = np.bincount(labels, minlength=CLS).astype(np.float32)
    safe = np.maximum(counts, 1.0)
    present = (counts > 0).astype(np.float32)
    valid = max(float(present.sum()), 1.0)

    onehot = np.zeros((B, CLS), dtype=np.float32)
    onehot[np.arange(B), labels] = 1.0
    oh_seg = onehot.astype(ml_dtypes.bfloat16)                      # [B, CLS]
    ohT = np.ascontiguousarray(onehot.T).astype(ml_dtypes.bfloat16)  # [CLS, B]

    thr = np.ascontiguousarray(dw[labels].reshape(NT, PT).T).astype(np.float32)
    w2 = np.ascontiguousarray(
        (1.0 / (CLS * safe[labels])).reshape(NT, PT).T
    ).astype(np.float32)
    a1 = (1.0 - ETA * present).reshape(CLS, 1).astype(np.float32)
    a3p = (ETA * present / safe).reshape(CLS, 1).astype(np.float32)
    # emas = (eta*p/cnt)*sums = eta*p*means -> gram_e = (eta*p)^2 * gram
    d2scale = (1.0 / np.maximum(ETA * present, 1e-6) ** 2).reshape(CLS)
    d2s128 = np.concatenate([d2scale, d2scale]).reshape(2 * CLS, 1).astype(
        np.float32
    )
    # packed-gram between-loss mask [128, 26]: present/(28*valid) on valid
    # pair slots, 0 on diag/dup/garbage slots
    maskpk = np.zeros((2 * CLS, _GCOLS), dtype=np.float32)
    for d, n, cb in _GJOBS:
        for k in range(n):
            if d >= 1:  # top block: pair (k, k+d); d=0 is the diag
                maskpk[0:CLS, cb + k] = present / (28.0 * valid)
            if k < 4 - d:  # bottom block: pair (k, k+d+4)
                maskpk[CLS : 2 * CLS, cb + k] = present / (28.0 * valid)
    ones_col = np.ones((PT, 1), dtype=np.float32)
    lab_i = np.ascontiguousarray(
        labels.reshape(NT, PT).T
    ).astype(np.int32)                                              # [PT, NT]
    warm = np.zeros((8, 32), dtype=ml_dtypes.bfloat16)

    import concourse.bacc as bacc

    nc = bacc.Bacc("TRN2", target_bir_lowering=False, num_devices=M)
    p_ext = nc.declare_dram_parameter("p", [B, H, FL], f32, isOutput=False)
    c_ext = nc.declare_dram_parameter("c", [CLS, H, FL], f32, isOutput=False)
    out_ext = nc.declare_dram_parameter("out", [1, 1], f32, isOutput=True)

    d_ohseg = nc.inline_tensor(oh_seg, "ohseg")
    d_ohT = nc.inline_tensor(ohT, "ohT")
    d_thr = nc.inline_tensor(thr, "thr")
    d_w2 = nc.inline_tensor(w2, "w2")
    d_a1 = nc.inline_tensor(a1, "a1c")
    d_a3p = nc.inline_tensor(a3p, "a3p")
    d_maskpk = nc.inline_tensor(maskpk, "maskpk")
    d_d2s = nc.inline_tensor(d2s128, "d2s")
    d_ones = nc.inline_tensor(ones_col, "onescol")
    d_lab = nc.inline_tensor(lab_i, "labi")
    d_warm = nc.inline_tensor(warm, "warm")

    with tile.TileContext(nc) as tc:
      try:
        with (
            tc.tile_pool(name="const", bufs=1) as constp,
            tc.tile_pool(name="pbp", bufs=1) as pbp,
            tc.tile_pool(name="ld", bufs=3) as ldp,
            tc.tile_pool(name="mid", bufs=1) as midp,
            tc.tile_pool(name="dram", bufs=1, space="DRAM") as dramp,
        ):
            # ---- warmup AllReduce: absorb the first-AR penalty ----
            warm_bin = dramp.tile([8, 32], bf16, name="warmbin")
            warm_bout = dramp.tile([8, 32], bf16, addr_space="Shared",
                                   name="warmbout")
            nc.scalar.dma_start(out=warm_bin[:], in_=d_warm[:])
            nc.gpsimd.collective_compute(
                "AllReduce", OP.add, RG,
                ins=[warm_bin.opt()], outs=[warm_bout.opt()],
            )

            # ---- P f32 loads on the sync hwdge queue, half-tiles ----
            pb = pbp.tile([PT, NT, H, FL], bf16)
            pview = p_ext[:].rearrange("(t p) h f -> t p h f", p=PT)
            pts = []
            for t in range(NT):
                ph = []
                for half in range(2):
                    pt_h = ldp.tile([PT, 4, FL], f32, tag="pt", bufs=4)
                    nc.sync.dma_start(
                        out=pt_h[:],
                        in_=pview[t][:, 4 * half : 4 * half + 4, :],
                    )
                    ph.append(pt_h)
                pts.append(ph)
            c_f = constp.tile([CLS, H, FL], f32)
            nc.scalar.dma_start(out=c_f[:], in_=c_ext[:])

            # ---- constants to SBUF (scalar hwdge queue) ----
            oh_sb = constp.tile([PT, NT, CLS], bf16)
            nc.scalar.dma_start(
                out=oh_sb[:],
                in_=d_ohseg[:].rearrange("(t p) c -> p t c", p=PT),
            )
            ohT_sb = constp.tile([CLS, NT * PT], bf16)
            nc.scalar.dma_start(out=ohT_sb[:], in_=d_ohT[:])
            thr_sb = constp.tile([PT, NT], f32)
            nc.scalar.dma_start(out=thr_sb[:], in_=d_thr[:])
            w2_sb = constp.tile([PT, NT], f32)
            nc.scalar.dma_start(out=w2_sb[:], in_=d_w2[:])
            a1_sb = constp.tile([CLS, 1], f32)
            nc.scalar.dma_start(out=a1_sb[:], in_=d_a1[:])
            a3p_sb = constp.tile([CLS, 1], f32)
            nc.scalar.dma_start(out=a3p_sb[:], in_=d_a3p[:])
            maskpk_sb = constp.tile([2 * CLS, _GCOLS], f32)
            nc.scalar.dma_start(out=maskpk_sb[:], in_=d_maskpk[:])
            d2s_sb = constp.tile([2 * CLS, 1], f32)
            nc.scalar.dma_start(out=d2s_sb[:], in_=d_d2s[:])
            ones_sb = constp.tile([PT, 1], f32)
            nc.scalar.dma_start(out=ones_sb[:], in_=d_ones[:])
            lab_sb = constp.tile([PT, NT], i32)
            nc.scalar.dma_start(out=lab_sb[:], in_=d_lab[:])

            # DVE tick absorbers
            absorb = midp.tile([PT, 8], f32)
            for i, src in enumerate(
                (
                    thr_sb[:, 0:1],
                    w2_sb[:, 0:1],
                    a1_sb[:, 0:1],
                    a3p_sb[:, 0:1],
                    maskpk_sb[:, 0:1],
                    d2s_sb[:, 0:1],
                )
            ):
                nc.vector.tensor_copy(
                    out=absorb[: src.shape[0], i : i + 1], in_=src
                )

            # ---- load-loop state ----
            sq_sb = midp.tile([PT, NT, H], f32)      # local sum of squares
            sqb = midp.tile([PT, NT * H], bf16)      # bf16 cast for AR1
            sqg = midp.tile([PT, NT * H], bf16)      # AR1 result
            sqf = midp.tile([PT, NT * H], f32)
            invps = midp.tile([PT, NT, H], f32)      # 1/||P[b,h]|| global
            ohn_sb = midp.tile([PT, NT, H, CLS], bf16)
            ipsh = midp.tile([PT, NT * H], f32)      # per-head inner prods

            sq_bin0 = dramp.tile([PT, 6 * H], bf16, name="sqbin0")
            sq_bout0 = dramp.tile([PT, 6 * H], bf16, addr_space="Shared",
                                  name="sqbout0")
            sq_bin1 = dramp.tile([PT, 2 * H], bf16, name="sqbin1")
            sq_bout1 = dramp.tile([PT, 2 * H], bf16, addr_space="Shared",
                                  name="sqbout1")

            with tc.tile_pool(name="psA", bufs=1, space="PSUM") as psA:
                ps_sums = psA.tile([CLS, H, FL], f32)

                for t in range(NT):
                    for half in range(2):
                        pt_h = pts[t][half]
                        hs = slice(4 * half, 4 * half + 4)
                        nc.scalar.activation(
                            out=pb[:, t, hs, :], in_=pt_h[:], func=AF.Copy
                        )
                    # sum of squares per head on DVE from the bf16 tile
                    sqd = ldp.tile([PT, H, FL], bf16, tag="sqd", bufs=1)
                    nc.vector.tensor_mul(
                        out=sqd[:], in0=pb[:, t], in1=pb[:, t]
                    )
                    nc.vector.tensor_reduce(
                        out=sq_sb[:, t, :],
                        in_=sqd[:],
                        axis=AX.X,
                        op=OP.add,
                    )
                    if t == 5:
                        nc.vector.tensor_copy(
                            out=sqb[:, 0 : 6 * H],
                            in_=sq_sb[:, 0:6, :].rearrange(
                                "p t h -> p (t h)"
                            ),
                        )
                        nc.sync.dma_start(
                            out=sq_bin0[:], in_=sqb[:, 0 : 6 * H]
                        )
                        nc.gpsimd.collective_compute(
                            "AllReduce", OP.add, RG,
                            ins=[sq_bin0.opt()],
                            outs=[sq_bout0.opt()],
                        )


                # chunk 1 (tiles 6-7) AR, then both chunk posts
                nc.vector.tensor_copy(
                    out=sqb[:, 6 * H :],
                    in_=sq_sb[:, 6:8, :].rearrange("p t h -> p (t h)"),
                )
                nc.sync.dma_start(
                    out=sq_bin1[:], in_=sqb[:, 6 * H :]
                )
                nc.gpsimd.collective_compute(
                    "AllReduce", OP.add, RG,
                    ins=[sq_bin1.opt()],
                    outs=[sq_bout1.opt()],
                )
                for lo, hi in ((0, 6), (6, NT)):
                    cols = slice(lo * H, hi * H)
                    bout = sq_bout0 if lo == 0 else sq_bout1
                    nc.sync.dma_start(
                        out=sqg[:, cols], in_=bout[:]
                    )
                    nc.vector.tensor_copy(out=sqf[:, cols], in_=sqg[:, cols])
                    nc.vector.reciprocal(out=sqf[:, cols], in_=sqf[:, cols])
                    nc.scalar.activation(
                        out=invps[:, lo:hi, :].rearrange("p t h -> p (t h)"),
                        in_=sqf[:, cols],
                        func=AF.Sqrt,
                    )
                    for t in range(lo, hi):
                        for h in range(H):
                            nc.vector.tensor_scalar(
                                out=ohn_sb[:, t, h, :],
                                in0=oh_sb[:, t, :],
                                scalar1=invps[:, t, h : h + 1],
                                scalar2=None,
                                op0=OP.mult,
                            )

                if stage <= 1:
                    nc.sync.dma_start(out=out_ext[:], in_=invps[0:1, 0, 0:1])
                    raise _StageCut()

                # ---- phase B h-major + per-half spine and gather table ----
                emasD = midp.tile([2 * CLS, H * FL], bf16)
                c_a1 = midp.tile([CLS, H * FL], bf16)
                nc.scalar.activation(
                    out=c_a1[:],
                    in_=c_f[:].rearrange("c h f -> c (h f)"),
                    func=AF.Copy,
                    scale=a1_sb[:],
                )
                cupd = c_a1  # in-place update below
                csq_sb = midp.tile([CLS, H], f32)
                csqb = midp.tile([CLS, H], bf16)
                csqsc = midp.tile([CLS, FL], bf16)
                tbls = [
                    dramp.tile([CLS, 4 * FL], bf16, name=f"cupdtbl{x}")
                    for x in range(2)
                ]
                for gi, heads in enumerate(_HALVES):
                    for h in heads:
                        for t in range(NT):
                            nc.tensor.matmul(
                                ps_sums[:, h, :],
                                lhsT=ohn_sb[:, t, h, :],
                                rhs=pb[:, t, h, :],
                                start=(t == 0),
                                stop=(t == NT - 1),
                            )
                    cs = slice(heads[0] * FL, (heads[-1] + 1) * FL)
                    nc.scalar.activation(
                        out=emasD[0:CLS, cs],
                        in_=ps_sums[:, heads[0] : heads[-1] + 1, :].rearrange(
                            "c h f -> c (h f)"
                        ),
                        func=AF.Copy,
                        scale=a3p_sb[:],
                    )
                    nc.vector.tensor_add(
                        out=cupd[:, cs], in0=c_a1[:, cs], in1=emasD[0:CLS, cs]
                    )
                    nc.sync.dma_start(out=tbls[gi][:], in_=cupd[:, cs])
                    for h in heads:
                        nc.scalar.activation(
                            out=csqsc[:],
                            in_=cupd[:, h * FL : (h + 1) * FL],
                            func=AF.Square,
                            accum_out=csq_sb[:, h : h + 1],
                        )
            # PSUM free from here on.
            nc.vector.tensor_copy(out=csqb[:], in_=csq_sb[:])
            ar2_bin = dramp.tile([CLS, H], bf16, name="ar2bin")
            ar2_bout = dramp.tile([CLS, H], bf16, addr_space="Shared",
                                  name="ar2bout")
            nc.sync.dma_start(out=ar2_bin[:], in_=csqb[:])

            if stage <= 2:
                nc.sync.dma_start(out=out_ext[:], in_=csq_sb[0:1, 0:1])
                raise _StageCut()

            # emas copies for the packed gram (DMA, overlap phase D-A)
            emasS = midp.tile([2 * CLS, H * FL], bf16)
            nc.sync.dma_start(
                out=emasD[CLS : 2 * CLS, :], in_=emasD[0:CLS, :]
            )
            nc.scalar.dma_start(out=emasS[0:CLS, :], in_=emasD[0:CLS, :])
            nc.sync.dma_start(
                out=emasS[CLS : 2 * CLS, 0 : 4 * FL],
                in_=emasD[0:CLS, 4 * FL : 8 * FL],
            )
            nc.scalar.dma_start(
                out=emasS[CLS : 2 * CLS, 4 * FL : 8 * FL],
                in_=emasD[0:CLS, 0 : 4 * FL],
            )

            # ---- phase D: interleaved A/B gathers, head slots split
            # across DVE (fused STT) / gpsimd (raw mul) / ACT (accum);
            # gram jobs woven in on DVE; csq AR rides the gpsimd queue ----
            sttsc = midp.tile([PT, FL], bf16)
            gram_pk = midp.tile([2 * CLS, _GCOLS], f32)
            ipw = midp.tile([PT, NT, H], f32)
            ips2 = midp.tile([PT, NT], f32)
            NA = 6   # ips2 tiles in the early AR chunk
            ar3a_bin = dramp.tile([PT, NA + _GCOLS], bf16, name="ar3abin")
            ar3a_bout = dramp.tile([PT, NA + _GCOLS], bf16,
                                   addr_space="Shared", name="ar3about")
            ar3b_bin = dramp.tile([PT, NT - NA], bf16, name="ar3bbin")
            ar3b_bout = dramp.tile([PT, NT - NA], bf16,
                                   addr_space="Shared", name="ar3bbout")
            packa = midp.tile([PT, NA + _GCOLS], bf16)
            packbb = midp.tile([PT, NT - NA], bf16)

            def issue_gathers(t):
                gA = ldp.tile([PT, 4, FL], bf16, tag="gA", bufs=3)
                nc.gpsimd.indirect_dma_start(
                    out=gA[:].rearrange("p h f -> p (h f)"),
                    out_offset=None,
                    in_=tbls[0][:],
                    in_offset=bass.IndirectOffsetOnAxis(
                        ap=lab_sb[:, t : t + 1], axis=0
                    ),
                )
                gB = ldp.tile([PT, 4, FL], bf16, tag="gB", bufs=3)
                nc.gpsimd.indirect_dma_start(
                    out=gB[:].rearrange("p h f -> p (h f)"),
                    out_offset=None,
                    in_=tbls[1][:],
                    in_offset=bass.IndirectOffsetOnAxis(
                        ap=lab_sb[:, t : t + 1], axis=0
                    ),
                )
                return gA, gB

            for t in range(NT):
                gA, gB = issue_gathers(t)
                for h in D_STT_DVE:
                    nc.vector.scalar_tensor_tensor(
                        out=sttsc[:],
                        in0=pb[:, t, h, :],
                        scalar=invps[:, t, h : h + 1],
                        in1=gA[:, h, :],
                        op0=OP.mult,
                        op1=OP.mult,
                        accum_out=ipsh[:, t * H + h : t * H + h + 1],
                    )
                for h in D_MUL_GPS:
                    dm = ldp.tile([PT, FL], bf16, tag="dm", bufs=12)
                    src = gA[:, h, :] if h < 4 else gB[:, h - 4, :]
                    nc.gpsimd.tensor_mul(
                        out=dm[:], in0=pb[:, t, h, :], in1=src
                    )
                    nc.scalar.activation(
                        out=dm[:],
                        in_=dm[:],
                        func=AF.Copy,
                        accum_out=ipsh[:, t * H + h : t * H + h + 1],
                    )
                for h in D_MUL_DVE:
                    dm = ldp.tile([PT, FL], bf16, tag="dm", bufs=12)
                    src = gA[:, h, :] if h < 4 else gB[:, h - 4, :]
                    nc.vector.tensor_mul(
                        out=dm[:], in0=pb[:, t, h, :], in1=src
                    )
                    nc.scalar.activation(
                        out=dm[:],
                        in_=dm[:],
                        func=AF.Copy,
                        accum_out=ipsh[:, t * H + h : t * H + h + 1],
                    )
                if t == 3:
                    # csq AllReduce trigger (bin ready once csqB is done)
                    nc.gpsimd.collective_compute(
                        "AllReduce", OP.add, RG,
                        ins=[ar2_bin.opt()], outs=[ar2_bout.opt()],
                    )
                if t == 5:
                    # invc / invcg / combine weights: issued mid-loop so the
                    # ACT Sqrt lands in the queue while AR2a is already done,
                    # instead of behind all remaining accums
                    csqg = midp.tile([CLS, H], bf16)
                    nc.sync.dma_start(out=csqg[:], in_=ar2_bout[:])
                    csqf = midp.tile([CLS, H], f32)
                    nc.vector.tensor_copy(out=csqf[:], in_=csqg[:])
                    nc.vector.reciprocal(out=csqf[:], in_=csqf[:])
                    invc_sb = midp.tile([CLS, H], bf16)
                    nc.scalar.activation(
                        out=invc_sb[:], in_=csqf[:], func=AF.Sqrt
                    )
                    invcg = midp.tile([PT, NT, H], f32)
                    with tc.tile_pool(name="psB", bufs=1, space="PSUM") as psB:
                        ps_icg = psB.tile([PT, NT, 512], f32)
                        for tt in range(NT):
                            nc.tensor.matmul(
                                ps_icg[:, tt, 0:H],
                                lhsT=ohT_sb[:, tt * PT : (tt + 1) * PT],
                                rhs=invc_sb[:],
                                start=True,
                                stop=True,
                                skip_group_check=True,
                            )
                        nc.vector.tensor_copy(
                            out=invcg[:], in_=ps_icg[:, :, 0:H]
                        )
                    wcomb = midp.tile([PT, NT, H], f32)
                    nc.vector.tensor_copy(out=wcomb[:], in_=invcg[:])
                    for hh in D_MUL_GPS + D_MUL_DVE:
                        nc.vector.tensor_mul(
                            out=wcomb[:, :, hh],
                            in0=invcg[:, :, hh],
                            in1=invps[:, :, hh],
                        )
                if 1 <= t <= 4:
                    d, n, cb = _GJOBS[t - 1]
                    gp = ldp.tile([2 * CLS, H, FL], bf16, tag="gp", bufs=1)
                    nc.vector.tensor_mul(
                        out=gp[:, 0:n, :].rearrange("p n f -> p (n f)"),
                        in0=emasD[:, 0 : n * FL],
                        in1=emasS[:, d * FL : (d + n) * FL],
                    )
                    nc.vector.tensor_reduce(
                        out=gram_pk[:, cb : cb + n],
                        in_=gp[:, 0:n, :],
                        axis=AX.X,
                        op=OP.add,
                    )
                if t == 5:
                    # early final-AR chunk: ips2 tiles 0-5 + gram
                    nc.vector.tensor_mul(
                        out=ipw[:, 0:NA, :].rearrange("p t h -> p (t h)"),
                        in0=ipsh[:, 0 : NA * H],
                        in1=wcomb[:, 0:NA, :].rearrange("p t h -> p (t h)"),
                    )
                    nc.vector.tensor_reduce(
                        out=ips2[:, 0:NA], in_=ipw[:, 0:NA, :],
                        axis=AX.X, op=OP.add,
                    )
                    nc.vector.tensor_copy(
                        out=packa[:, 0:NA], in_=ips2[:, 0:NA]
                    )
                    nc.vector.tensor_copy(
                        out=packa[:, NA : NA + _GCOLS], in_=gram_pk[:]
                    )
                    nc.sync.dma_start(out=ar3a_bin[:], in_=packa[:])
                if t == 6:
                    nc.gpsimd.collective_compute(
                        "AllReduce", OP.add, RG,
                        ins=[ar3a_bin.opt()], outs=[ar3a_bout.opt()],
                    )


            if stage <= 3:
                nc.sync.dma_start(out=out_ext[:], in_=invcg[0:1, 0, 0:1])
                raise _StageCut()

            # ---- late final-AR chunk: ips2 tiles 6-7 ----
            nc.vector.tensor_mul(
                out=ipw[:, NA:, :].rearrange("p t h -> p (t h)"),
                in0=ipsh[:, NA * H :],
                in1=wcomb[:, NA:, :].rearrange("p t h -> p (t h)"),
            )
            nc.vector.tensor_reduce(
                out=ips2[:, NA:], in_=ipw[:, NA:, :], axis=AX.X, op=OP.add
            )
            nc.vector.tensor_copy(out=packbb[:], in_=ips2[:, NA:])
            nc.sync.dma_start(out=ar3b_bin[:], in_=packbb[:])
            nc.gpsimd.collective_compute(
                "AllReduce", OP.add, RG,
                ins=[ar3b_bin.opt()], outs=[ar3b_bout.opt()],
            )
            packg = midp.tile([PT, NA + _GCOLS], bf16)
            nc.sync.dma_start(out=packg[:], in_=ar3a_bout[:])
            packg_b = midp.tile([PT, NT - NA], bf16)
            nc.sync.dma_start(out=packg_b[:], in_=ar3b_bout[:])

            if stage <= 4:
                ipsdbg = midp.tile([1, 1], f32)
                nc.vector.tensor_copy(out=ipsdbg[:], in_=packg[0:1, 0:1])
                nc.sync.dma_start(out=out_ext[:], in_=ipsdbg[:])
                raise _StageCut()

            # ---- within-loss: dist = sqrt(16-2ips) ----
            dst = midp.tile([PT, NT], f32)
            nc.vector.tensor_copy(out=dst[:, 0:NA], in_=packg[:, 0:NA])
            nc.vector.tensor_copy(out=dst[:, NA:], in_=packg_b[:])
            nc.vector.tensor_scalar(
                out=dst[:],
                in0=dst[:],
                scalar1=-2.0,
                scalar2=16.0,
                op0=OP.mult,
                op1=OP.add,
            )
            nc.vector.tensor_scalar_max(out=dst[:], in0=dst[:], scalar1=0.0)
            nc.scalar.activation(out=dst[:], in_=dst[:], func=AF.Sqrt)
            rr = midp.tile([PT, NT], f32)
            nc.vector.tensor_sub(out=rr[:], in0=dst[:], in1=thr_sb[:])
            nc.vector.tensor_scalar_max(out=rr[:], in0=rr[:], scalar1=0.0)
            wdump = midp.tile([PT, NT], f32)
            wcol = midp.tile([PT, 1], f32)
            nc.vector.tensor_mul(out=wdump[:], in0=rr[:], in1=w2_sb[:])
            nc.vector.tensor_reduce(
                out=wcol[:], in_=wdump[:], axis=AX.X, op=OP.add
            )

            if stage <= 5:
                nc.sync.dma_start(out=out_ext[:], in_=wcol[0:1, 0:1])
                raise _StageCut()

            # ---- between-loss from packed gram ----
            gramf = midp.tile([2 * CLS, _GCOLS], f32)
            nc.vector.tensor_copy(
                out=gramf[:], in_=packg[:, NA : NA + _GCOLS]
            )
            # sqm (mean-norm^2) lives in top J0 cols 0:8; scale by d2s
            sqA = midp.tile([2 * CLS, H], f32)
            sqS = midp.tile([2 * CLS, H], f32)
            nc.vector.tensor_scalar(
                out=sqA[0:CLS, :],
                in0=gramf[0:CLS, 0:H],
                scalar1=d2s_sb[0:CLS],
                scalar2=None,
                op0=OP.mult,
            )
            # replicate to bottom block + rolled copy via tiny SBUF DMAs
            nc.sync.dma_start(out=sqA[CLS : 2 * CLS, :], in_=sqA[0:CLS, :])
            nc.scalar.dma_start(out=sqS[0:CLS, :], in_=sqA[0:CLS, :])
            nc.sync.dma_start(
                out=sqS[CLS : 2 * CLS, 0:4], in_=sqA[0:CLS, 4:8]
            )
            nc.scalar.dma_start(
                out=sqS[CLS : 2 * CLS, 4:8], in_=sqA[0:CLS, 0:4]
            )
            d2 = midp.tile([2 * CLS, _GCOLS], f32)
            for d, n, cb in _GJOBS:
                nc.vector.tensor_add(
                    out=d2[:, cb : cb + n],
                    in0=sqA[:, 0:n],
                    in1=sqS[:, d : d + n],
                )
            gm2 = midp.tile([2 * CLS, _GCOLS], f32)
            nc.vector.tensor_scalar(
                out=gm2[:],
                in0=gramf[:],
                scalar1=d2s_sb[:],
                scalar2=-2.0,
                op0=OP.mult,
                op1=OP.mult,
            )
            nc.vector.tensor_add(out=d2[:], in0=d2[:], in1=gm2[:])
            nc.vector.tensor_scalar_max(out=d2[:], in0=d2[:], scalar1=1e-12)
            nc.scalar.activation(out=d2[:], in_=d2[:], func=AF.Sqrt)
            lb = midp.tile([2 * CLS, _GCOLS], f32)
            nc.scalar.activation(
                out=lb[:], in_=d2[:], func=AF.Relu, bias=DELTA_BETWEEN,
                scale=-1.0,
            )
            bdump = midp.tile([2 * CLS, _GCOLS], f32)
            bcol = midp.tile([2 * CLS, 1], f32)
            nc.vector.tensor_mul(out=bdump[:], in0=lb[:], in1=maskpk_sb[:])
            nc.vector.tensor_reduce(
                out=bcol[:], in_=bdump[:], axis=AX.X, op=OP.add
            )

            if stage <= 6:
                nc.sync.dma_start(out=out_ext[:], in_=bcol[0:1, 0:1])
                raise _StageCut()

            # ---- final partition reduction via ones-matmul ----
            fcol = midp.tile([PT, 1], f32)
            nc.vector.tensor_add(out=fcol[:], in0=wcol[:], in1=bcol[:])
            res = midp.tile([1, 1], f32)
            with tc.tile_pool(name="psC", bufs=1, space="PSUM") as psC:
                fin = psC.tile([1, 1], f32)
                nc.tensor.matmul(
                    fin[:],
                    lhsT=ones_sb[:],
                    rhs=fcol[:],
                    start=True,
                    stop=True,
                    skip_group_check=True,
                )
                nc.vector.tensor_copy(out=res[:], in_=fin[:])
            nc.sync.dma_start(out=out_ext[:], in_=res[:])

      except _StageCut:
        pass

    if not nc.is_finalized():
        nc.finalize()
    return nc


def _install_ntff_shim():
    """The agent image's antenv lacks axon_hooks; synthesize it so
    run_bass_kernel_spmd(trace=True) can capture an NTFF profile."""
    import types

    if "antenv.axon_hooks" in sys.modules:
        return
    try:
        from trn_agent_boot.trn_boot import _ntff_profile_via_ctypes
    except ImportError:
        return
    hook = _ntff_profile_via_ctypes("/opt/axon/libaxon_pjrt.so")
    if hook is None:
        return
    mod = types.ModuleType("antenv.axon_hooks")
    _state = {"hook": hook}
    mod.set_axon_ntff_profile_hook = lambda h: _state.__setitem__("hook", h)
    mod.get_axon_ntff_profile_hook = lambda: _state["hook"]
    sys.modules["antenv.axon_hooks"] = mod
    import antenv

    antenv.axon_hooks = mod


def kernel(P, labels, C, delta_within, stage=99):
    global LAST_EXEC_NS, LAST_RESULTS
    P = np.asarray(P, dtype=np.float32)
    C = np.asarray(C, dtype=np.float32)

    nc = _build(labels, delta_within, stage=stage)

    in_maps = []
    for i in range(M):
        sl = slice(i * FL, (i + 1) * FL)
        in_maps.append(
            {
                "p": np.ascontiguousarray(P[:, :, sl]),
                "c": np.ascontiguousarray(C[:, :, sl]),
            }
        )

    from concourse import bass_utils

    if TRACE:
        _install_ntff_shim()

    res = bass_utils.run_bass_kernel_spmd(
        nc, in_maps, core_ids=list(range(M)), trace=TRACE
    )
    LAST_EXEC_NS = res.exec_time_ns
    LAST_RESULTS = res
    if TRACE and res.exec_time_ns is not None:
        times = [res.exec_time_ns]
        for _ in range(2):
            r2 = bass_utils.run_bass_kernel_spmd(
                nc, in_maps, core_ids=list(range(M)), trace=True
            )
            if r2.exec_time_ns is not None:
                times.append(r2.exec_time_ns)
        print(f"exec times: {times}")
        LAST_EXEC_NS = min(times)
    out = np.asarray(res.results[0]["out"], dtype=np.float32).reshape(())
    return out


# revision 56
# speedup vs baseline: 1.1080x; 1.1080x over previous
"""Distributed Trainium2 kernel for nn_ADLoss_38354057953796 (v5).

Strategy: shard P and C along the FRAME axis (F=4096 -> 512 per core).
Each core sees the full batch for its frame slice, so per-class segment
sums are local PE matmuls; only tiny AllReduces cross cores.

v5 (from v4 trace analysis, 267us):
  * The CC stack has a fixed startup barrier (~45-58us starting at
    ~21us) that gates the first collective, and ARs serialize at
    ~8us each on one CC stream.  Chunked AR1 therefore only added
    serial CC time; v5 uses ONE warmup (absorbs the first-AR penalty
    during the load) + ONE 16KB AR1 of all per-(b,h) sq norms.
  * P stays RAW in SBUF (bf16); invn is folded into the phase-B lhsT
    (ohn = onehot*invn) and into phase D fused ops / combine weights.
  * Phase B runs h-major and the RAW C_upd gather table is written in
    TWO head-halves, so phase D-A (heads 0-3) starts while phase B is
    still finishing heads 4-7.
  * Phase D head slots split by measured cost: STT on DVE (~0.95us),
    raw mul on gpsimd (~1.36us) or DVE (~0.49us) + accum on ACT
    (~0.99us).  gpsimd scalar_tensor_tensor is broken on this stack
    (device hang) - only plain tensor ops there.
  * gram: 4 packed jobs (128 partitions, bottom block = emas rolled by
    4 heads), DVE muls + DVE job-reduces, issued between the A and B
    phase-D loops.
  * Collectives: warmup, AR1 (sq), AR2a (csq 1KB), AR3 (ips2+gram in
    one 8.7KB bin).
"""

import sys
import numpy as np

for _p in ("/opt/trn_rl_repo",):
    if _p not in sys.path:
        sys.path.insert(0, _p)

B, H, F, CLS = 1024, 8, 4096, 64
M = 8            # cores
FL = F // M      # local frame slice = 512
NT = 8           # batch tiles
PT = 128         # partitions per tile
ETA = 0.1
DELTA_BETWEEN = 1.0

TRACE = False
LAST_EXEC_NS = None
LAST_RESULTS = None

# packed gram jobs: (d_top, d_bottom=d+4) pairs; bottom uses emas rolled
# by 4 heads
_GJOBS = [(0, 8, 0), (1, 7, 8), (2, 6, 15), (3, 5, 21)]  # (d, n, col_base)
_GCOLS = 26

# phase-D head slots: heads 0-3 gather from table A, 4-7 from table B
D_STT_DVE = (0, 1, 2)        # fused (pb*invn)*g + accum on DVE
D_MUL_GPS = (3, 4, 5)        # raw mul on gpsimd, accum on ACT
D_MUL_DVE = (6, 7)           # raw mul on DVE, accum on ACT
_HALVES = ((0, 1, 2, 3), (4, 5, 6, 7))


class _StageCut(Exception):
    pass


def _build(labels, delta_within, stage=99):
    import concourse.bass as bass
    import concourse.tile as tile
    from concourse import mybir
    import ml_dtypes

    f32 = mybir.dt.float32
    bf16 = mybir.dt.bfloat16
    i32 = mybir.dt.int32
    AF = mybir.ActivationFunctionType
    OP = mybir.AluOpType
    AX = mybir.AxisListType
    RG = [list(range(M))]

    labels = np.asarray(labels).astype(np.int64).reshape(B)
    dw = np.asarray(delta_within).astype(np.float32).reshape(CLS)

    counts = np.bincount(labels, minlength=CLS).astype(np.float32)
    safe = np.maximum(counts, 1.0)
    present = (counts > 0).astype(np.float32)
    valid = max(float(present.sum()), 1.0)

    onehot = np.zeros((B, CLS), dtype=np.float32)
    onehot[np.arange(B), labels] = 1.0
    oh_seg = onehot.astype(ml_dtypes.bfloat16)                      # [B, CLS]
    ohT = np.ascontiguousarray(onehot.T).astype(ml_dtypes.bfloat16)  # [CLS, B]

    thr = np.ascontiguousarray(dw[labels].reshape(NT, PT).T).astype(np.float32)
    w2 = np.ascontiguousarray(
        (1.0 / (CLS * safe[labels])).reshape(NT, PT).T
    ).astype(np.float32)
    a1 = (1.0 - ETA * present).reshape(CLS, 1).astype(np.float32)
    a3p = (ETA * present / safe).reshape(CLS, 1).astype(np.float32)
    # emas = (eta*p/cnt)*sums = eta*p*means -> gram_e = (eta*p)^2 * gram
    d2scale = (1.0 / np.maximum(ETA * present, 1e-6) ** 2).reshape(CLS)
    d2s128 = np.concatenate([d2scale, d2scale]).reshape(2 * CLS, 1).astype(
        np.float32
    )
    # packed-gram between-loss mask [128, 26]: present/(28*valid) on valid
    # pair slots, 0 on diag/dup/garbage slots
    maskpk = np.zeros((2 * CLS, _GCOLS), dtype=np.float32)
    for d, n, cb in _GJOBS:
        for k in range(n):
            if d >= 1:  # top block: pair (k, k+d); d=0 is the diag
                maskpk[0:CLS, cb + k] = present / (28.0 * valid)
            if k < 4 - d:  # bottom block: pair (k, k+d+4)
                maskpk[CLS : 2 * CLS, cb + k] = present / (28.0 * valid)
    ones_col = np.ones((PT, 1), dtype=np.float32)
    lab_i = np.ascontiguousarray(
        labels.reshape(NT, PT).T
    ).astype(np.int32)                                              # [PT, NT]
    warm = np.zeros((8, 32), dtype=ml_dtypes.bfloat16)

    import concourse.bacc as bacc

    nc = bacc.Bacc("TRN2", target_bir_lowering=False, num_devices=M)
    p_ext = nc.declare_dram_parameter("p", [B, H, FL], f32, isOutput=False)
    c_ext = nc.declare_dram_parameter("c", [CLS, H, FL], f32, isOutput=False)
    out_ext = nc.declare_dram_parameter("out", [1, 1], f32, isOutput=True)

    d_ohseg = nc.inline_tensor(oh_seg, "ohseg")
    d_ohT = nc.inline_tensor(ohT, "ohT")
    d_thr = nc.inline_tensor(thr, "thr")
    d_w2 = nc.inline_tensor(w2, "w2")
    d_a1 = nc.inline_tensor(a1, "a1c")
    d_a3p = nc.inline_tensor(a3p, "a3p")
    d_maskpk = nc.inline_tensor(maskpk, "maskpk")
    d_d2s = nc.inline_tensor(d2s128, "d2s")
    d_ones = nc.inline_tensor(ones_col, "onescol")
    d_lab = nc.inline_tensor(lab_i, "labi")
    d_warm = nc.inline_tensor(warm, "warm")

    with tile.TileContext(nc) as tc:
      try:
        with (
            tc.tile_pool(name="const", bufs=1) as constp,
            tc.tile_pool(name="pbp", bufs=1) as pbp,
            tc.tile_pool(name="ld", bufs=3) as ldp,
            tc.tile_pool(name="mid", bufs=1) as midp,
            tc.tile_pool(name="dram", bufs=1, space="DRAM") as dramp,
        ):
            # ---- warmup AllReduce: absorb the first-AR penalty ----
            warm_bin = dramp.tile([8, 32], bf16, name="warmbin")
            warm_bout = dramp.tile([8, 32], bf16, addr_space="Shared",
                                   name="warmbout")
            nc.scalar.dma_start(out=warm_bin[:], in_=d_warm[:])
            nc.gpsimd.collective_compute(
                "AllReduce", OP.add, RG,
                ins=[warm_bin.opt()], outs=[warm_bout.opt()],
            )

            # ---- P f32 loads on the sync hwdge queue, half-tiles ----
            pb = pbp.tile([PT, NT, H, FL], bf16)
            pview = p_ext[:].rearrange("(t p) h f -> t p h f", p=PT)
            pts = []
            for t in range(NT):
                ph = []
                for half in range(2):
                    pt_h = ldp.tile([PT, 4, FL], f32, tag="pt", bufs=4)
                    nc.sync.dma_start(
                        out=pt_h[:],
                        in_=pview[t][:, 4 * half : 4 * half + 4, :],
                    )
                    ph.append(pt_h)
                pts.append(ph)
            c_f = constp.tile([CLS, H, FL], f32)
            nc.scalar.dma_start(out=c_f[:], in_=c_ext[:])

            # ---- constants to SBUF (scalar hwdge queue) ----
            oh_sb = constp.tile([PT, NT, CLS], bf16)
            nc.scalar.dma_start(
                out=oh_sb[:],
                in_=d_ohseg[:].rearrange("(t p) c -> p t c", p=PT),
            )
            ohT_sb = constp.tile([CLS, NT * PT], bf16)
            nc.scalar.dma_start(out=ohT_sb[:], in_=d_ohT[:])
            thr_sb = constp.tile([PT, NT], f32)
            nc.scalar.dma_start(out=thr_sb[:], in_=d_thr[:])
            w2_sb = constp.tile([PT, NT], f32)
            nc.scalar.dma_start(out=w2_sb[:], in_=d_w2[:])
            a1_sb = constp.tile([CLS, 1], f32)
            nc.scalar.dma_start(out=a1_sb[:], in_=d_a1[:])
            a3p_sb = constp.tile([CLS, 1], f32)
            nc.scalar.dma_start(out=a3p_sb[:], in_=d_a3p[:])
            maskpk_sb = constp.tile([2 * CLS, _GCOLS], f32)
            nc.scalar.dma_start(out=maskpk_sb[:], in_=d_maskpk[:])
            d2s_sb = constp.tile([2 * CLS, 1], f32)
            nc.scalar.dma_start(out=d2s_sb[:], in_=d_d2s[:])
            ones_sb = constp.tile([PT, 1], f32)
            nc.scalar.dma_start(out=ones_sb[:], in_=d_ones[:])
            lab_sb = constp.tile([PT, NT], i32)
            nc.scalar.dma_start(out=lab_sb[:], in_=d_lab[:])

            # DVE tick absorbers
            absorb = midp.tile([PT, 8], f32)
            for i, src in enumerate(
                (
                    thr_sb[:, 0:1],
                    w2_sb[:, 0:1],
                    a1_sb[:, 0:1],
                    a3p_sb[:, 0:1],
                    maskpk_sb[:, 0:1],
                    d2s_sb[:, 0:1],
                )
            ):
                nc.vector.tensor_copy(
                    out=absorb[: src.shape[0], i : i + 1], in_=src
                )

            # ---- load-loop state ----
            sq_sb = midp.tile([PT, NT, H], f32)      # local sum of squares
            sqb = midp.tile([PT, NT * H], bf16)      # bf16 cast for AR1
            sqg = midp.tile([PT, NT * H], bf16)      # AR1 result
            sqf = midp.tile([PT, NT * H], f32)
            invps = midp.tile([PT, NT, H], f32)      # 1/||P[b,h]|| global
            ohn_sb = midp.tile([PT, NT, H, CLS], bf16)
            ipsh = midp.tile([PT, NT * H], f32)      # per-head inner prods

            sq_bin0 = dramp.tile([PT, 6 * H], bf16, name="sqbin0")
            sq_bout0 = dramp.tile([PT, 6 * H], bf16, addr_space="Shared",
                                  name="sqbout0")
            sq_bin1 = dramp.tile([PT, 2 * H], bf16, name="sqbin1")
            sq_bout1 = dramp.tile([PT, 2 * H], bf16, addr_space="Shared",
                                  name="sqbout1")

            with tc.tile_pool(name="psA", bufs=1, space="PSUM") as psA:
                ps_sums = psA.tile([CLS, H, FL], f32)

                for t in range(NT):
                    for half in range(2):
                        pt_h = pts[t][half]
                        hs = slice(4 * half, 4 * half + 4)
                        nc.scalar.activation(
                            out=pb[:, t, hs, :], in_=pt_h[:], func=AF.Copy
                        )
                    # sum of squares per head on DVE from the bf16 tile
                    sqd = ldp.tile([PT, H, FL], bf16, tag="sqd", bufs=1)
                    nc.vector.tensor_mul(
                        out=sqd[:], in0=pb[:, t], in1=pb[:, t]
                    )
                    nc.vector.tensor_reduce(
                        out=sq_sb[:, t, :],
                        in_=sqd[:],
                        axis=AX.X,
                        op=OP.add,
                    )
                    if t == 5:
                        nc.vector.tensor_copy(
                            out=sqb[:, 0 : 6 * H],
                            in_=sq_sb[:, 0:6, :].rearrange(
                                "p t h -> p (t h)"
                            ),
                        )
                        nc.sync.dma_start(
                            out=sq_bin0[:], in_=sqb[:, 0 : 6 * H]
                        )
                        nc.gpsimd.collective_compute(
                            "AllReduce", OP.add, RG,
                            ins=[sq_bin0.opt()],
                            outs=[sq_bout0.opt()],
                        )


                # chunk 1 (tiles 6-7) AR, then both chunk posts
                nc.vector.tensor_copy(
                    out=sqb[:, 6 * H :],
                    in_=sq_sb[:, 6:8, :].rearrange("p t h -> p (t h)"),
                )
                nc.sync.dma_start(
                    out=sq_bin1[:], in_=sqb[:, 6 * H :]
                )
                nc.gpsimd.collective_compute(
                    "AllReduce", OP.add, RG,
                    ins=[sq_bin1.opt()],
                    outs=[sq_bout1.opt()],
                )
                for lo, hi in ((0, 6), (6, NT)):
                    cols = slice(lo * H, hi * H)
                    bout = sq_bout0 if lo == 0 else sq_bout1
                    nc.sync.dma_start(
                        out=sqg[:, cols], in_=bout[:]
                    )
                    nc.vector.tensor_copy(out=sqf[:, cols], in_=sqg[:, cols])
                    nc.vector.reciprocal(out=sqf[:, cols], in_=sqf[:, cols])
                    nc.scalar.activation(
                        out=invps[:, lo:hi, :].rearrange("p t h -> p (t h)"),
                        in_=sqf[:, cols],
                        func=AF.Sqrt,
                    )
                    for t in range(lo, hi):
                        for h in range(H):
                            nc.vector.tensor_scalar(
                                out=ohn_sb[:, t, h, :],
                                in0=oh_sb[:, t, :],
                                scalar1=invps[:, t, h : h + 1],
                                scalar2=None,
                                op0=OP.mult,
                            )

                if stage <= 1:
                    nc.sync.dma_start(out=out_ext[:], in_=invps[0:1, 0, 0:1])
                    raise _StageCut()

                # ---- phase B h-major + per-half spine and gather table ----
                emasD = midp.tile([2 * CLS, H * FL], bf16)
                c_a1 = midp.tile([CLS, H * FL], bf16)
                nc.scalar.activation(
                    out=c_a1[:],
                    in_=c_f[:].rearrange("c h f -> c (h f)"),
                    func=AF.Copy,
                    scale=a1_sb[:],
                )
                cupd = c_a1  # in-place update below
                csq_sb = midp.tile([CLS, H], f32)
                csqb = midp.tile([CLS, H], bf16)
                csqsc = midp.tile([CLS, FL], bf16)
                tbls = [
                    dramp.tile([CLS, 4 * FL], bf16, name=f"cupdtbl{x}")
                    for x in range(2)
                ]
                for gi, heads in enumerate(_HALVES):
                    for h in heads:
                        for t in range(NT):
                            nc.tensor.matmul(
                                ps_sums[:, h, :],
                                lhsT=ohn_sb[:, t, h, :],
                                rhs=pb[:, t, h, :],
                                start=(t == 0),
                                stop=(t == NT - 1),
                            )
                    cs = slice(heads[0] * FL, (heads[-1] + 1) * FL)
                    nc.scalar.activation(
                        out=emasD[0:CLS, cs],
                        in_=ps_sums[:, heads[0] : heads[-1] + 1, :].rearrange(
                            "c h f -> c (h f)"
                        ),
                        func=AF.Copy,
                        scale=a3p_sb[:],
                    )
                    nc.vector.tensor_add(
                        out=cupd[:, cs], in0=c_a1[:, cs], in1=emasD[0:CLS, cs]
                    )
                    nc.sync.dma_start(out=tbls[gi][:], in_=cupd[:, cs])
                    for h in heads:
                        nc.scalar.activation(
                            out=csqsc[:],
                            in_=cupd[:, h * FL : (h + 1) * FL],
                            func=AF.Square,
                            accum_out=csq_sb[:, h : h + 1],
                        )
            # PSUM free from here on.
            nc.vector.tensor_copy(out=csqb[:], in_=csq_sb[:])
            ar2_bin = dramp.tile([CLS, H], bf16, name="ar2bin")
            ar2_bout = dramp.tile([CLS, H], bf16, addr_space="Shared",
                                  name="ar2bout")
            nc.sync.dma_start(out=ar2_bin[:], in_=csqb[:])

            if stage <= 2:
                nc.sync.dma_start(out=out_ext[:], in_=csq_sb[0:1, 0:1])
                raise _StageCut()

            # emas copies for the packed gram (DMA, overlap phase D-A)
            emasS = midp.tile([2 * CLS, H * FL], bf16)
            nc.sync.dma_start(
                out=emasD[CLS : 2 * CLS, :], in_=emasD[0:CLS, :]
            )
            nc.scalar.dma_start(out=emasS[0:CLS, :], in_=emasD[0:CLS, :])
            nc.sync.dma_start(
                out=emasS[CLS : 2 * CLS, 0 : 4 * FL],
                in_=emasD[0:CLS, 4 * FL : 8 * FL],
            )
            nc.scalar.dma_start(
                out=emasS[CLS : 2 * CLS, 4 * FL : 8 * FL],
                in_=emasD[0:CLS, 0 : 4 * FL],
            )

            # ---- phase D: interleaved A/B gathers, head slots split
            # across DVE (fused STT) / gpsimd (raw mul) / ACT (accum);
            # gram jobs woven in on DVE; csq AR rides the gpsimd queue ----
            sttsc = midp.tile([PT, FL], bf16)
            gram_pk = midp.tile([2 * CLS, _GCOLS], f32)

            def issue_gathers(t):
                gA = ldp.tile([PT, 4, FL], bf16, tag="gA", bufs=3)
                nc.gpsimd.indirect_dma_start(
                    out=gA[:].rearrange("p h f -> p (h f)"),
                    out_offset=None,
                    in_=tbls[0][:],
                    in_offset=bass.IndirectOffsetOnAxis(
                        ap=lab_sb[:, t : t + 1], axis=0
                    ),
                )
                gB = ldp.tile([PT, 4, FL], bf16, tag="gB", bufs=3)
                nc.gpsimd.indirect_dma_start(
                    out=gB[:].rearrange("p h f -> p (h f)"),
                    out_offset=None,
                    in_=tbls[1][:],
                    in_offset=bass.IndirectOffsetOnAxis(
                        ap=lab_sb[:, t : t + 1], axis=0
                    ),
                )
                return gA, gB

            for t in range(NT):
                gA, gB = issue_gathers(t)
                for h in D_STT_DVE:
                    nc.vector.scalar_tensor_tensor(
                        out=sttsc[:],
                        in0=pb[:, t, h, :],
                        scalar=invps[:, t, h : h + 1],
                        in1=gA[:, h, :],
                        op0=OP.mult,
                        op1=OP.mult,
                        accum_out=ipsh[:, t * H + h : t * H + h + 1],
                    )
                for h in D_MUL_GPS:
                    dm = ldp.tile([PT, FL], bf16, tag="dm", bufs=12)
                    src = gA[:, h, :] if h < 4 else gB[:, h - 4, :]
                    nc.gpsimd.tensor_mul(
                        out=dm[:], in0=pb[:, t, h, :], in1=src
                    )
                    nc.scalar.activation(
                        out=dm[:],
                        in_=dm[:],
                        func=AF.Copy,
                        accum_out=ipsh[:, t * H + h : t * H + h + 1],
                    )
                for h in D_MUL_DVE:
                    dm = ldp.tile([PT, FL], bf16, tag="dm", bufs=12)
                    src = gA[:, h, :] if h < 4 else gB[:, h - 4, :]
                    nc.vector.tensor_mul(
                        out=dm[:], in0=pb[:, t, h, :], in1=src
                    )
                    nc.scalar.activation(
                        out=dm[:],
                        in_=dm[:],
                        func=AF.Copy,
                        accum_out=ipsh[:, t * H + h : t * H + h + 1],
                    )
                if t == 3:
                    # csq AllReduce trigger (bin ready once csqB is done)
                    nc.gpsimd.collective_compute(
                        "AllReduce", OP.add, RG,
                        ins=[ar2_bin.opt()], outs=[ar2_bout.opt()],
                    )
                if t == 5:
                    # invc / invcg / combine weights: issued mid-loop so the
                    # ACT Sqrt lands in the queue while AR2a is already done,
                    # instead of behind all remaining accums
                    csqg = midp.tile([CLS, H], bf16)
                    nc.sync.dma_start(out=csqg[:], in_=ar2_bout[:])
                    csqf = midp.tile([CLS, H], f32)
                    nc.vector.tensor_copy(out=csqf[:], in_=csqg[:])
                    nc.vector.reciprocal(out=csqf[:], in_=csqf[:])
                    invc_sb = midp.tile([CLS, H], bf16)
                    nc.scalar.activation(
                        out=invc_sb[:], in_=csqf[:], func=AF.Sqrt
                    )
                    invcg = midp.tile([PT, NT, H], f32)
                    with tc.tile_pool(name="psB", bufs=1, space="PSUM") as psB:
                        ps_icg = psB.tile([PT, NT, 512], f32)
                        for tt in range(NT):
                            nc.tensor.matmul(
                                ps_icg[:, tt, 0:H],
                                lhsT=ohT_sb[:, tt * PT : (tt + 1) * PT],
                                rhs=invc_sb[:],
                                start=True,
                                stop=True,
                                skip_group_check=True,
                            )
                        nc.vector.tensor_copy(
                            out=invcg[:], in_=ps_icg[:, :, 0:H]
                        )
                    wcomb = midp.tile([PT, NT, H], f32)
                    nc.vector.tensor_copy(out=wcomb[:], in_=invcg[:])
                    for hh in D_MUL_GPS + D_MUL_DVE:
                        nc.vector.tensor_mul(
                            out=wcomb[:, :, hh],
                            in0=invcg[:, :, hh],
                            in1=invps[:, :, hh],
                        )
                if t % 2 == 1:
                    d, n, cb = _GJOBS[(t - 1) // 2]
                    gp = ldp.tile([2 * CLS, H, FL], bf16, tag="gp", bufs=1)
                    nc.vector.tensor_mul(
                        out=gp[:, 0:n, :].rearrange("p n f -> p (n f)"),
                        in0=emasD[:, 0 : n * FL],
                        in1=emasS[:, d * FL : (d + n) * FL],
                    )
                    nc.vector.tensor_reduce(
                        out=gram_pk[:, cb : cb + n],
                        in_=gp[:, 0:n, :],
                        axis=AX.X,
                        op=OP.add,
                    )


            if stage <= 3:
                nc.sync.dma_start(out=out_ext[:], in_=invcg[0:1, 0, 0:1])
                raise _StageCut()

            # ---- ips2 = sum_h ipsh*wcomb; ONE final AR (ips2 + gram) ----
            ipw = midp.tile([PT, NT, H], f32)
            nc.vector.tensor_mul(
                out=ipw[:].rearrange("p t h -> p (t h)"),
                in0=ipsh[:],
                in1=wcomb[:].rearrange("p t h -> p (t h)"),
            )
            ips2 = midp.tile([PT, NT], f32)
            nc.vector.tensor_reduce(
                out=ips2[:], in_=ipw[:], axis=AX.X, op=OP.add
            )
            ar3_bin = dramp.tile([PT, NT + _GCOLS], bf16, name="ar3bin")
            ar3_bout = dramp.tile([PT, NT + _GCOLS], bf16,
                                  addr_space="Shared", name="ar3bout")
            packb = midp.tile([PT, NT + _GCOLS], bf16)
            nc.vector.tensor_copy(out=packb[:, NT : NT + _GCOLS],
                                  in_=gram_pk[:])
            nc.vector.tensor_copy(out=packb[:, 0:NT], in_=ips2[:])
            nc.sync.dma_start(out=ar3_bin[:], in_=packb[:])
            nc.gpsimd.collective_compute(
                "AllReduce", OP.add, RG,
                ins=[ar3_bin.opt()], outs=[ar3_bout.opt()],
            )
            packg = midp.tile([PT, NT + _GCOLS], bf16)
            nc.sync.dma_start(out=packg[:], in_=ar3_bout[:])

            if stage <= 4:
                ipsdbg = midp.tile([1, 1], f32)
                nc.vector.tensor_copy(out=ipsdbg[:], in_=packg[0:1, 0:1])
                nc.sync.dma_start(out=out_ext[:], in_=ipsdbg[:])
                raise _StageCut()

            # ---- within-loss: dist = sqrt(16-2ips) ----
            dst = midp.tile([PT, NT], f32)
            nc.vector.tensor_copy(out=dst[:], in_=packg[:, 0:NT])
            nc.vector.tensor_scalar(
                out=dst[:],
                in0=dst[:],
                scalar1=-2.0,
                scalar2=16.0,
                op0=OP.mult,
                op1=OP.add,
            )
            nc.vector.tensor_scalar_max(out=dst[:], in0=dst[:], scalar1=0.0)
            nc.scalar.activation(out=dst[:], in_=dst[:], func=AF.Sqrt)
            rr = midp.tile([PT, NT], f32)
            nc.vector.tensor_sub(out=rr[:], in0=dst[:], in1=thr_sb[:])
            nc.vector.tensor_scalar_max(out=rr[:], in0=rr[:], scalar1=0.0)
            wdump = midp.tile([PT, NT], f32)
            wcol = midp.tile([PT, 1], f32)
            nc.vector.tensor_mul(out=wdump[:], in0=rr[:], in1=w2_sb[:])
            nc.vector.tensor_reduce(
                out=wcol[:], in_=wdump[:], axis=AX.X, op=OP.add
            )

            if stage <= 5:
                nc.sync.dma_start(out=out_ext[:], in_=wcol[0:1, 0:1])
                raise _StageCut()

            # ---- between-loss from packed gram ----
            gramf = midp.tile([2 * CLS, _GCOLS], f32)
            nc.vector.tensor_copy(
                out=gramf[:], in_=packg[:, NT : NT + _GCOLS]
            )
            # sqm (mean-norm^2) lives in top J0 cols 0:8; scale by d2s
            sqA = midp.tile([2 * CLS, H], f32)
            sqS = midp.tile([2 * CLS, H], f32)
            nc.vector.tensor_scalar(
                out=sqA[0:CLS, :],
                in0=gramf[0:CLS, 0:H],
                scalar1=d2s_sb[0:CLS],
                scalar2=None,
                op0=OP.mult,
            )
            # replicate to bottom block + rolled copy via tiny SBUF DMAs
            nc.sync.dma_start(out=sqA[CLS : 2 * CLS, :], in_=sqA[0:CLS, :])
            nc.scalar.dma_start(out=sqS[0:CLS, :], in_=sqA[0:CLS, :])
            nc.sync.dma_start(
                out=sqS[CLS : 2 * CLS, 0:4], in_=sqA[0:CLS, 4:8]
            )
            nc.scalar.dma_start(
                out=sqS[CLS : 2 * CLS, 4:8], in_=sqA[0:CLS, 0:4]
            )
            d2 = midp.tile([2 * CLS, _GCOLS], f32)
            for d, n, cb in _GJOBS:
                nc.vector.tensor_add(
                    out=d2[:, cb : cb + n],
                    in0=sqA[:, 0:n],
                    in1=sqS[:, d : d + n],
                )
            gm2 = midp.tile([2 * CLS, _GCOLS], f32)
            nc.vector.tensor_scalar(
                out=gm2[:],
                in0=gramf[:],
                scalar1=d2s_sb[:],
                scalar2=-2.0,
                op0=OP.mult,
                op1=OP.mult,
            )
            nc.vector.tensor_add(out=d2[:], in0=d2[:], in1=gm2[:])
            nc.vector.tensor_scalar_max(out=d2[:], in0=d2[:], scalar1=1e-12)
            nc.scalar.activation(out=d2[:], in_=d2[:], func=AF.Sqrt)
            lb = midp.tile([2 * CLS, _GCOLS], f32)
            nc.scalar.activation(
                out=lb[:], in_=d2[:], func=AF.Relu, bias=DELTA_BETWEEN,
                scale=-1.0,
            )
            bdump = midp.tile([2 * CLS, _GCOLS], f32)
            bcol = midp.tile([2 * CLS, 1], f32)
            nc.vector.tensor_mul(out=bdump[:], in0=lb[:], in1=maskpk_sb[:])
            nc.vector.tensor_reduce(
                out=bcol[:], in_=bdump[:], axis=AX.X, op=OP.add
            )

            if stage <= 6:
                nc.sync.dma_start(out=out_ext[:], in_=bcol[0:1, 0:1])
                raise _StageCut()

            # ---- final partition reduction via ones-matmul ----
            fcol = midp.tile([PT, 1], f32)
            nc.vector.tensor_add(out=fcol[:], in0=wcol[:], in1=bcol[:])
            res = midp.tile([1, 1], f32)
            with tc.tile_pool(name="psC", bufs=1, space="PSUM") as psC:
                fin = psC.tile([1, 1], f32)
                nc.tensor.matmul(
                    fin[:],
                    lhsT=ones_sb[:],
                    rhs=fcol[:],
                    start=True,
                    stop=True,
                    skip_group_check=True,
                )
                nc.vector.tensor_copy(out=res[:], in_=fin[:])
            nc.sync.dma_start(out=out_ext[:], in_=res[:])

      except _StageCut:
        pass

    if not nc.is_finalized():
        nc.finalize()
    return nc


def _install_ntff_shim():
    """The agent image's antenv lacks axon_hooks; synthesize it so
    run_bass_kernel_spmd(trace=True) can capture an NTFF profile."""
    import types

    if "antenv.axon_hooks" in sys.modules:
        return
    try:
        from trn_agent_boot.trn_boot import _ntff_profile_via_ctypes
    except ImportError:
        return
    hook = _ntff_profile_via_ctypes("/opt/axon/libaxon_pjrt.so")
    if hook is None:
        return
    mod = types.ModuleType("antenv.axon_hooks")
    _state = {"hook": hook}
    mod.set_axon_ntff_profile_hook = lambda h: _state.__setitem__("hook", h)
    mod.get_axon_ntff_profile_hook = lambda: _state["hook"]
    sys.modules["antenv.axon_hooks"] = mod
    import antenv

    antenv.axon_hooks = mod


def kernel(P, labels, C, delta_within, stage=99):
    global LAST_EXEC_NS, LAST_RESULTS
    P = np.asarray(P, dtype=np.float32)
    C = np.asarray(C, dtype=np.float32)

    nc = _build(labels, delta_within, stage=stage)

    in_maps = []
    for i in range(M):
        sl = slice(i * FL, (i + 1) * FL)
        in_maps.append(
            {
                "p": np.ascontiguousarray(P[:, :, sl]),
                "c": np.ascontiguousarray(C[:, :, sl]),
            }
        )

    from concourse import bass_utils

    if TRACE:
        _install_ntff_shim()

    res = bass_utils.run_bass_kernel_spmd(
        nc, in_maps, core_ids=list(range(M)), trace=TRACE
    )
    LAST_EXEC_NS = res.exec_time_ns
    LAST_RESULTS = res
    if TRACE and res.exec_time_ns is not None:
        times = [res.exec_time_ns]
        for _ in range(2):
            r2 = bass_utils.run_bass_kernel_spmd(
                nc, in_maps, core_ids=list(range(M)), trace=True
            )
            if r2.exec_time_ns is not None:
                times.append(r2.exec_time_ns)
        print(f"exec times: {times}")
        LAST_EXEC_NS = min(times)
    out = np.asarray(res.results[0]["out"], dtype=np.float32).reshape(())
    return out
